# revision 1
# baseline (speedup 1.0000x reference)
"""Multi-head self-attention (B=2, N=4096, C=384, H=6) on 8 Trainium2 NeuronCores.

Sharding: core = (batch, query-quarter). Each core recomputes K/V for its batch
from x (no cross-core communication), computes Q for its 1024 query rows, runs
softmax(Q K^T / sqrt(D)) V for all 6 heads, and applies the output projection
for its rows. Host concatenates the 8 row-slices.

Device layout notes:
  - Everything is kept "transposed" (channel on partitions) so the PE never
    needs an on-chip transpose: S^T = (K^T)^T-matmul and O^T = V^T-matmul come
    out directly in the layout the next matmul wants.
  - All attention operands are bf16 (x, weights, K^T, Q^T, V, exp(S), scores
    in PSUM): the PE streams 2 bf16 cols/cycle and FWL doubles weight loads,
    so the matmul side runs ~2x vs fp32r. Accumulation stays fp32 in PSUM.
  - Score matmuls cover the core's full 1024 queries in one op (N=1024 bf16
    out = exactly one PSUM bank); the head-pair's two K=64 matmuls are
    row-tiled (partition base 0/64) so they stream concurrently.
  - exp() runs on one [128, 2048] PSUM tile per k-tile (both heads x 1024
    queries): 96 big ACT ops/core instead of 384 small ones keeps the
    scalar engine (the ~165us/core exp roofline) streaming.
  - V carries an appended ones-column per head, so the P@V matmul also
    produces the softmax denominator for free (row 64 of O^T).
  - softmax skips the max-subtraction: scores are ~N(0,1), exp can't overflow.
  - O accumulates in two [65, 1024] fp32 PSUM tiles per pair (4 banks);
    with the double-buffered score tiles (2x2 banks) PSUM is exactly full.
    At pair end O is copied to SBUF so the normalization chain (reciprocal +
    partition-broadcast via a DRAM bounce + multiply) runs entirely off
    PSUM/PE while the next pair's matmuls already reuse the banks.
"""

import numpy as np
import ml_dtypes
from contextlib import ExitStack

import concourse.bass as bass
import concourse.bacc as bacc
import concourse.tile as tile
from concourse import mybir
from concourse.bass_utils import run_bass_kernel_spmd

B, N, C = 2, 4096, 384
H, D = 6, 64
SCALE = D ** -0.5
P = 128
QPC = 1024          # query rows per core
NCORES = 8
PAIRS = H // 2      # 3 head pairs
NT = N // P         # 32 k-tiles
BF = mybir.dt.bfloat16
F32 = mybir.dt.float32
MDT = mybir.dt.float32r
I16 = mybir.dt.int16
EXP = mybir.ActivationFunctionType.Exp
NPBF = ml_dtypes.bfloat16
# Schraudolph fast-exp constants, bf16 flavor:
# exp(x) ~= bitcast_bf16(int16(x*EXPA + EXPB)). EXPB is calibrated for
# truncation (the DVE fp32->int16 convert) with the corrector 0.0440
# minimizing max relative error (~3.3%) over the score range.
EXPA = float(2 ** 7 / np.log(2.0))
EXPB = float((127 - 0.0440) * 2 ** 7 + 0.5)
# Which exp units (unit = 2*kt + head_half, mod 16) run on DVE instead of
# ACT: 8/16 is the ACT/DVE balance point (DVE pays a PSUM-read penalty but
# also carries the normalization). Units 0/1 (first k-tile of each 8-kt
# block) go to DVE: the pair-boundary O copy runs on ACT, so DVE-first
# keeps both engines busy through the boundary.
DVE_UNITS = frozenset({0, 1, 3, 5, 7, 9, 11, 13})
# flash-lite: pair 0's first FLASH_KT k-tiles run their score matmuls + exp
# inside the kvgen x-stream (one k-tile per 512-key chunk, which is always
# already K-copied), filling the otherwise idle exp engines; their PV
# matmuls catch up once the O PSUM banks free up after kvgen.
FLASH_KT = 14


def _emit(ctx: ExitStack, tc, nc, xT, xqT, wq, wk, wv, wp, bias, out):
    sing = ctx.enter_context(tc.tile_pool(name="sing", bufs=1))
    ktp = ctx.enter_context(tc.tile_pool(name="ktp", bufs=1))
    vp = ctx.enter_context(tc.tile_pool(name="vp", bufs=1))
    prep = ctx.enter_context(tc.tile_pool(name="prep", bufs=1))
    outp = ctx.enter_context(tc.tile_pool(name="outp", bufs=6))
    drp = ctx.enter_context(tc.tile_pool(name="drp", bufs=4, space="DRAM"))
    # PSUM pools are phase-scoped (stack discipline): kvgen/Q-gen use psp,
    # attention uses ssp (scores) + osp (O accumulators), proj re-creates
    # one. ssp is created FIRST so psp (stack top) can pop after kvgen
    # while ssp lives on through the flash-lite frontend into attention.
    actx = ExitStack()
    ssp = actx.enter_context(tc.tile_pool(name="ssp", bufs=2, space="PSUM"))
    pctx = ExitStack()
    psp = pctx.enter_context(tc.tile_pool(name="psp", bufs=4, space="PSUM"))
    # created last / released right after Q^T generation (stack discipline)
    qctx = ExitStack()
    qtmp = qctx.enter_context(tc.tile_pool(name="qtmp", bufs=1))

    # ---- load weights / per-core query slice ----
    wq_sb = qtmp.tile([P, 3, C], BF, name="wq_sb")
    wk_sb = sing.tile([P, 3, C], BF, name="wk_sb")
    wv_sb = sing.tile([P, 3, C], BF, name="wv_sb")
    wp_sb = sing.tile([64, H, C], BF, name="wp_sb")
    xq_sb = qtmp.tile([P, 3, QPC], BF, name="xq_sb")
    # Q-gen's operands first on the SP queue; the other weights ride the
    # (idle at t=0) ACT/DVE DGE queues so the startup DMAs issue in parallel
    nc.sync.dma_start(out=wq_sb, in_=wq[:, :, :])
    # split so Q-gen's first matmuls start after the first half lands
    xq_r = xqT[:, :].rearrange("(ck p) q -> p ck q", p=P)
    nc.sync.dma_start(out=xq_sb[:, :, 0:512], in_=xq_r[:, :, 0:512])
    nc.sync.dma_start(out=xq_sb[:, :, 512:1024], in_=xq_r[:, :, 512:1024])
    nc.sync.dma_start(out=wk_sb, in_=wk[:, :, :])
    nc.sync.dma_start(out=wv_sb, in_=wv[:, :, :])
    nc.sync.dma_start(out=wp_sb, in_=wp[:, :, :])

    ones_nt = sing.tile([P, NT, 1], BF, name="ones_nt")
    nc.vector.memset(ones_nt, 1.0)
    ones64 = sing.tile([1, 64], BF, name="ones64")
    nc.vector.memset(ones64, 1.0)

    # dummy exp: pulls the ~1.3us exp table load off the first real exp's
    # critical path; reads an SBUF constant so it needs no DMA at all
    dume = sing.tile([P, 1], F32, name="dume")
    nc.scalar.activation(dume, ones_nt[:, 0, :], EXP)

    # PE warmup: ~5us of dependency-free matmuls during the initial DMA
    # wait ramp the tensor engine out of its cold p-state before Q-gen
    warm = sing.tile([1, 512], BF, name="warm")
    nc.vector.memset(warm, 0.0)
    for _ in range(6):
        wps = psp.tile([P, 512], F32, name="ps")[0:64, :]
        nc.tensor.matmul(wps, lhsT=ones64, rhs=warm, start=True, stop=True)

    # ---- Q^T for all pairs: QT[:, pair, q] = (wq_pair)^T @ xq ----
    qt_sb = sing.tile([P, PAIRS, QPC], BF, name="qt_sb")
    for pair in range(PAIRS):
        for qt in range(QPC // 512):
            ps = psp.tile([P, 512], F32, name="ps")
            for ck in range(3):
                nc.tensor.matmul(
                    ps,
                    lhsT=wq_sb[:, ck, pair * 128:(pair + 1) * 128],
                    rhs=xq_sb[:, ck, qt * 512:(qt + 1) * 512],
                    start=(ck == 0),
                    stop=(ck == 2),
                )
            nc.scalar.copy(qt_sb[:, pair, qt * 512:(qt + 1) * 512], ps)
    qctx.close()  # xq SBUF space is no longer needed
    # these pools reuse the released qtmp space (created after the pop so the
    # stack allocator can place them there): deeper buffering for the xT
    # stream, exp output, and the normalization chain
    xchp = ctx.enter_context(tc.tile_pool(name="xchp", bufs=3))
    # deep enough to hold the flash-lite frontend's exp tiles (2*FLASH_KT)
    # until the PV catch-up consumes them, plus steady-state slack
    expp = ctx.enter_context(tc.tile_pool(name="expp", bufs=2 * FLASH_KT + 4))
    # lifetimes: pair p's o_sb/rb tiles die at pair p+1's kt=7, before that
    # pair's own are created, so single/double buffering suffices
    rbp = ctx.enter_context(tc.tile_pool(name="rbp", bufs=1))
    onp = ctx.enter_context(tc.tile_pool(name="onp", bufs=1))

    pre = [prep.tile([64, QPC], BF, name=f"pre{h}") for h in range(H)]

    xT_r = xT[:, :].rearrange("(ck p) n -> p ck n", p=P)

    # ---- V for ALL pairs (with ones columns), one xT streaming pass
    # (rhs free dim 384 keeps one matmul per chunk; per-pair N=128 would
    # waste streams) ----
    v_all = vp.tile([P, PAIRS, NT, 130], BF, name="v_all")
    for p in range(PAIRS):
        nc.vector.tensor_copy(v_all[:, p, :, 64:65], ones_nt)
        nc.vector.tensor_copy(v_all[:, p, :, 129:130], ones_nt)

    def s_tiles(pair, kt_t, kt):
        # scores for both heads of the pair, kt's 128 keys x all 1024
        # queries: two [128,1024] fp32 PSUM tiles (fp32 matmul out caps
        # N at 512, so 2 matmuls per tile). The four matmuls are emitted
        # even/odd interleaved so adjacent K=64 matmuls sit on row
        # groups 0/64 and stream concurrently in the PE array.
        ts = [ssp.tile([P, QPC], F32, name="s") for _ in range(2)]
        for qh in range(2):
            for hh in range(2):
                lo = 64 * hh
                nc.tensor.matmul(
                    ts[hh][:, qh * 512:(qh + 1) * 512],
                    lhsT=kt_t[lo:lo + 64, kt * 128:(kt + 1) * 128],
                    rhs=qt_sb[lo:lo + 64, pair, qh * 512:(qh + 1) * 512],
                    start=True,
                    stop=True,
                )
        return ts

    def exp_unit(kt, hh, s_t):
        # exp on ACT (exact, spline) or DVE (Schraudolph bit-trick, ~3%
        # elementwise, ~1e-2 after softmax at this mix) so the two engines
        # stream the softmax concurrently. One shared int16 tile name (the
        # ACT path writes through a bf16 bitcast view) keeps the pool's
        # per-buffer footprint at a single tile.
        e_t = expp.tile([P, QPC], I16, name="e")
        if (2 * kt + hh) % 16 in DVE_UNITS:
            nc.vector.tensor_scalar(
                e_t, s_t, EXPA, EXPB,
                mybir.AluOpType.mult, mybir.AluOpType.add,
            )
        else:
            nc.scalar.activation(e_t[:, :].bitcast(BF), s_t, EXP)
        return lambda lo, hi: e_t[:, lo:hi].bitcast(BF)

    flash_rhs = []

    def kvgen(kt_tiles):
        """One xT streaming pass computing V (all pairs) and K^T (all pairs).
        PE-bound, so prefetch hides the DMA. After each chunk, one flash-lite
        frontend k-tile (scores + exp for pair 0) keeps the exp engines fed."""
        for nt8 in range(N // 512):
            xch = xchp.tile([P, 3, 512], BF, name="xch")
            nc.sync.dma_start(out=xch, in_=xT_r[:, :, nt8 * 512:(nt8 + 1) * 512])
            for sub in range(4):
                nt = nt8 * 4 + sub
                psv = psp.tile([P, 512], F32, name="ps")[:, 0:C]
                for ck in range(3):
                    nc.tensor.matmul(
                        psv,
                        lhsT=xch[:, ck, sub * 128:(sub + 1) * 128],
                        rhs=wv_sb[:, ck, :],
                        start=(ck == 0),
                        stop=(ck == 2),
                    )
                # one 4D copy moves all 3 pairs' V slices at once (the
                # per-pair version cost 3 DVE ops + their SEQ overhead)
                nc.vector.tensor_copy(
                    v_all[:, :, nt, :].rearrange(
                        "p pr (two y) -> p pr two y", two=2
                    )[:, :, :, 0:64],
                    psv[:, :].rearrange(
                        "p (pr two x) -> p pr two x", pr=PAIRS, two=2
                    ),
                )
            for p in range(PAIRS):
                ps = psp.tile([P, 512], F32, name="ps")
                for ck in range(3):
                    nc.tensor.matmul(
                        ps,
                        lhsT=wk_sb[:, ck, p * 128:(p + 1) * 128],
                        rhs=xch[:, ck, :],
                        start=(ck == 0),
                        stop=(ck == 2),
                    )
                # ACT is idle during kvgen; putting the K^T copies there
                # keeps DVE (the V copies) off the critical path
                nc.scalar.copy(kt_tiles[p][:, nt8 * 512:(nt8 + 1) * 512], ps)
            # flash-lite frontend: up to two ready k-tiles of pair 0 per chunk
            for _ in range(2):
                fkt = len(flash_rhs)
                if fkt >= FLASH_KT or fkt >= (nt8 + 1) * 4:
                    break
                f_e, f_o = s_tiles(0, kt_tiles[0], fkt)
                flash_rhs.append(
                    (exp_unit(fkt, 0, f_e), exp_unit(fkt, 1, f_o))
                )

    def attention(pair, kt_t, deferred):
        v_t = v_all[:, pair]
        o_t = osp.tile([65, 2, QPC], F32, name="o")  # [.., 0, ..]=even head

        def pv(kt, hh, e_rhs):
            base = 65 * hh
            for qh in range(2):
                nc.tensor.matmul(
                    o_t[:, hh, qh * 512:(qh + 1) * 512],
                    lhsT=v_t[:, kt, base:base + 65],
                    rhs=e_rhs(qh * 512, (qh + 1) * 512),
                    start=(kt == 0),
                    stop=(kt == NT - 1),
                )

        # The exp engines alternate the two ssp slots (even/odd head) while
        # the PE refills the just-freed slot within the other head's exp op.
        # Deferred normalization work from the previous pair is sprinkled at
        # kt milestones so the in-order DVE queue never waits on the
        # broadcast DMA. Pair 0's first FLASH_KT k-tiles were exp'ed during
        # kvgen; only their PV catch-up runs here.
        kt0 = FLASH_KT if pair == 0 else 0
        for kt in range(kt0):
            pv(kt, 0, flash_rhs[kt][0])
            pv(kt, 1, flash_rhs[kt][1])
        s_e, s_o = s_tiles(pair, kt_t, kt0)
        for kt in range(kt0, NT):
            while deferred and deferred[0][0] <= kt:
                deferred.pop(0)[1]()
            rhs_e = exp_unit(kt, 0, s_e)
            rhs_o = exp_unit(kt, 1, s_o)
            if kt + 1 < NT:
                s_e, s_o = s_tiles(pair, kt_t, kt + 1)
            pv(kt, 0, rhs_e)
            pv(kt, 1, rhs_o)
        # move O off PSUM immediately (the single osp buffer is reused by
        # the next pair's first PV); the rest of the normalization chain is
        # deferred into the next pair's stream.
        if pair < PAIRS - 1:
            o_sb = onp.tile([65, 2, QPC], F32, name="osb")
            # ACT (the lighter exp engine) takes the copy so the DVE can
            # start the next pair's exp units immediately
            nc.scalar.copy(o_sb, o_t)

            # The exact DVE reciprocal runs ~6 cyc/elem on ONE lane for a
            # [1,1024] row; bouncing the denominators across 64 partitions
            # via two tiny SBUF->SBUF DMAs spreads it over 64 lanes
            # (6.4us -> ~0.15us of DVE). Broadcast and multiply run on the
            # otherwise-idle GpSimd engine. The spread DMAs issue early
            # (kt=1) and the DVE reciprocals late (kt=3) so the in-order
            # DVE queue never waits on a DMA in flight.
            d64s = [rbp.tile([64, 16], F32, name=f"d64{hh}") for hh in range(2)]

            def norm_dmas():
                for hh in range(2):
                    nc.sync.dma_start(out=d64s[hh], in_=o_sb[64:65, hh, :])

            def norm_chain():
                for hh in range(2):
                    r64 = rbp.tile([64, 16], F32, name=f"r64{hh}")
                    nc.vector.reciprocal(r64, d64s[hh])
                    recip = rbp.tile([1, QPC], F32, name="recip")
                    nc.sync.dma_start(out=recip, in_=r64)
                    rb_sb = rbp.tile([64, QPC], F32, name=f"rb{hh}")
                    nc.gpsimd.partition_broadcast(rb_sb[:, :], recip[:, :])
                    nc.gpsimd.tensor_mul(
                        pre[pair * 2 + hh], o_sb[0:64, hh, :], rb_sb
                    )

            return [(1, norm_dmas), (3, norm_chain)]
        else:
            # last pair: only move O off PSUM here (on ACT, idle after its
            # last exp); the normalization itself runs after the attention
            # PSUM pools close, interleaved with the projection head-0..3
            # matmuls (see the tail section below).
            o_fl = onp.tile([65, 2, QPC], F32, name="osb")
            for qh in range(2):
                nc.scalar.copy(
                    o_fl[:, :, qh * 512:(qh + 1) * 512],
                    o_t[:, :, qh * 512:(qh + 1) * 512],
                )
            last_o.append(o_fl)
        return []

    kt_tiles = [ktp.tile([P, N], BF, name=f"kt{pair}") for pair in range(PAIRS)]
    # ssp coexists with the kvgen psp (4+4 banks) so the flash-lite frontend
    # can run scores+exp during the x stream; osp takes psp's banks after.
    kvgen(kt_tiles)
    pctx.close()  # free kvgen PSUM banks for the O accumulators

    # bias is first needed by proj ~200us from now; loading it here keeps
    # the startup DMA chain (which gates Q-gen) one transfer shorter
    bias_bc = sing.tile([P, C], F32, name="bias_bc")
    b_ap = bias[:, :]
    nc.sync.dma_start(
        out=bias_bc, in_=bass.AP(b_ap.tensor, b_ap.offset, [[0, P], [1, C]])
    )
    osp = actx.enter_context(tc.tile_pool(name="osp", bufs=1, space="PSUM"))
    deferred = []
    last_o = []
    for pair in range(PAIRS):
        deferred = attention(pair, kt_tiles[pair], deferred)
    actx.close()

    # tail pools: tailp (banks 0-3) holds the normalization broadcasts so
    # psp2 lands on the O-accumulator banks, whose last reader (the ACT
    # copies above) finishes earliest — the proj head-0..3 matmuls below
    # can then pre-run while the normalization chains are still going.
    tailp = ctx.enter_context(tc.tile_pool(name="tailp", bufs=4, space="PSUM"))
    psp = ctx.enter_context(tc.tile_pool(name="psp2", bufs=4, space="PSUM"))
    o_fl = last_o[0]

    # ---- output projection, interleaved with the last pair's norm ----
    # pass 1: heads 0-3 for the first four query chunks (ready immediately)
    ps_qc = []
    for qc in range(4):
        ps = psp.tile([P, 512], F32, name="ps")[:, 0:C]
        ps_qc.append(ps)
        for h in range(4):
            nc.tensor.matmul(
                ps,
                lhsT=pre[h][:, qc * P:(qc + 1) * P],
                rhs=wp_sb[:, h, :],
                start=(h == 0),
                stop=False,
            )
    # last pair's normalization: reciprocal (DVE) -> K=1 broadcast matmul
    # (PE, into tailp) -> multiply (DVE), per q-half and head
    for qh in range(2):
        for hh in range(2):
            h = 4 + hh
            recip = rbp.tile([1, 512], BF, name="recipl")
            with nc.allow_low_precision(
                reason="softmax denominator broadcast; bf16 ~0.4% "
                "is well inside the error budget"
            ):
                nc.vector.reciprocal(
                    recip, o_fl[64:65, hh, qh * 512:(qh + 1) * 512]
                )
            rb_ps = tailp.tile([64, 512], F32, name="rbps")
            nc.tensor.matmul(
                rb_ps, lhsT=ones64, rhs=recip, start=True, stop=True
            )
            nc.vector.tensor_mul(
                pre[h][:, qh * 512:(qh + 1) * 512],
                o_fl[0:64, hh, qh * 512:(qh + 1) * 512],
                rb_ps,
            )
    # pass 2: finish qc 0-3 with heads 4-5, then qc 4-7 in full
    for qc in range(QPC // P):
        if qc < 4:
            ps = ps_qc[qc]
            heads = range(4, H)
        else:
            ps = psp.tile([P, 512], F32, name="ps")[:, 0:C]
            heads = range(H)
        for h in heads:
            nc.tensor.matmul(
                ps,
                lhsT=pre[h][:, qc * P:(qc + 1) * P],
                rhs=wp_sb[:, h, :],
                start=(h == 0),
                stop=(h == H - 1),
            )
        o_sb = outp.tile([P, C], F32, name="osb")
        nc.vector.tensor_add(o_sb, ps, bias_bc)
        nc.sync.dma_start(out=out[qc * P:(qc + 1) * P, :], in_=o_sb)


def build_nc(reps=1):
    nc = bacc.Bacc()
    xT = nc.dram_tensor("xT", [C, N], BF, kind="ExternalInput")
    xqT = nc.dram_tensor("xqT", [C, QPC], BF, kind="ExternalInput")
    wq = nc.dram_tensor("wq", [P, 3, C], BF, kind="ExternalInput")
    wk = nc.dram_tensor("wk", [P, 3, C], BF, kind="ExternalInput")
    wv = nc.dram_tensor("wv", [P, 3, C], BF, kind="ExternalInput")
    wp = nc.dram_tensor("wp", [64, H, C], BF, kind="ExternalInput")
    bias = nc.dram_tensor("bias", [1, C], F32, kind="ExternalInput")
    out = nc.dram_tensor("out", [QPC, C], F32, kind="ExternalOutput")
    with tile.TileContext(nc) as tc:
        with ExitStack() as ctx:
            if reps == 1:
                _emit(ctx, tc, nc, xT, xqT, wq, wk, wv, wp, bias, out)
            else:
                # benchmark-only loop: branch-prefetch hints for the engines
                # whose bodies exceed one IRAM block, so the back-edge
                # I$-miss doesn't inflate the slope
                with tc.For_i(
                    0, reps, 1,
                    hint_engines=(mybir.EngineType.PE, mybir.EngineType.Activation),
                ):
                    _emit(ctx, tc, nc, xT, xqT, wq, wk, wv, wp, bias, out)
    nc.compile()
    return nc


_NC = None


def _get_nc():
    global _NC
    if _NC is None:
        _NC = build_nc()
    return _NC


def make_in_maps(x, w_qkv, w_proj, b_proj):
    x = np.asarray(x, np.float32)
    w_qkv = np.asarray(w_qkv, np.float32)
    w_proj = np.asarray(w_proj, np.float32)
    b_proj = np.asarray(b_proj, np.float32)

    wq = np.ascontiguousarray(
        (w_qkv[:, 0:C] * SCALE).reshape(3, P, C).transpose(1, 0, 2)
    ).astype(NPBF)
    wk = np.ascontiguousarray(
        w_qkv[:, C:2 * C].reshape(3, P, C).transpose(1, 0, 2)
    ).astype(NPBF)
    wv = np.ascontiguousarray(
        w_qkv[:, 2 * C:3 * C].reshape(3, P, C).transpose(1, 0, 2)
    ).astype(NPBF)
    wp = np.ascontiguousarray(w_proj.reshape(H, D, C).transpose(1, 0, 2)).astype(NPBF)
    bias = np.ascontiguousarray(b_proj.reshape(1, C))

    in_maps = []
    for core in range(NCORES):
        b, qi = core // 4, core % 4
        xT = np.ascontiguousarray(x[b].T).astype(NPBF)
        xq = np.ascontiguousarray(xT[:, qi * QPC:(qi + 1) * QPC])
        in_maps.append(
            {"xT": xT, "xqT": xq, "wq": wq, "wk": wk, "wv": wv, "wp": wp,
             "bias": bias}
        )
    return in_maps


def run(x, w_qkv, w_proj, b_proj, **run_kwargs):
    nc = _get_nc()
    in_maps = make_in_maps(x, w_qkv, w_proj, b_proj)
    res = run_bass_kernel_spmd(nc, in_maps, core_ids=list(range(NCORES)), **run_kwargs)
    out = np.empty((B, N, C), np.float32)
    for core in range(NCORES):
        b, qi = core // 4, core % 4
        out[b, qi * QPC:(qi + 1) * QPC] = res.results[core]["out"]
    return out, res


def kernel(x, w_qkv, w_proj, b_proj):
    out, _ = run(x, w_qkv, w_proj, b_proj)
    return out

